# revision 31
# baseline (speedup 1.0000x reference)
"""CompressedKVCache kernel for Trainium2 (8 NeuronCores, head-sharded).

Per (b, h) head: quantize k/v rows to int4 (per-row min/max affine),
then return the dequantized cache prefix [0, start+L): rows [0, start)
decoded from the packed uint8 cache inputs, rows [start, start+L)
quantize->dequantized entirely on-chip.

Sharding: H=32 heads split across 8 cores (4 heads each); fully
independent per head, no cross-core communication.

Layout: row-block-per-partition ("(p c) d") so every DMA descriptor is
1-8KB contiguous (a "(c p) d" layout produces 512B descriptors and
leaves the DMA engines descriptor-rate-bound at ~47% HBM utilization).

Engine split (measured per-op costs):
  DVE   - min/max 3D reduces, stats chain, nibble unpack, 11/16 of the
          quant-region dequant chunks (tensor_scalar, ~314ns/chunk)
  ACT   - all quantize chunks (Identity, scale/bias APs, i32 out = RNE
          round for free; 387ns/chunk) + 5/16 of dequant chunks
  GPSIMD- all prefix dequant chunks (u8 in, strided interleave out,
          ~445ns/chunk). NOTE: gpsimd mis-executes i32 inputs; only u8
          inputs are used here.

Work is pipelined per tensor-unit (B*HC*2 = 16 units of 2048 rows):
unit u's dequant is emitted during unit u+1 so no engine stalls on the
ACT quant stream; prefix outputs DMA out as soon as GPSIMD finishes.
Prefix scale/zero rows are hoisted into one DMA per input tensor.
"""

import sys

sys.path.insert(0, "/opt/trn_rl_repo")

import numpy as np
from concourse import bass, mybir
from concourse import tile
from concourse.bass_utils import run_bass_kernel_spmd

F32 = mybir.dt.float32
U8 = mybir.dt.uint8
U32 = mybir.dt.uint32
I32 = mybir.dt.int32
Alu = mybir.AluOpType
Act = mybir.ActivationFunctionType
AX = mybir.AxisListType
INV15 = float(np.float32(1.0 / 15.0))

B, H, L, D = 2, 32, 2048, 128
MAX_SEQ = 8192
N_CORES = 8
HC = H // N_CORES  # heads per core
CQ = L // 128      # quant row-chunks per head (16)

# Per-unit dequant chunk split across engines (of CQ=16 chunks):
# first ACT_DEQ on ACT, next GPS_DEQ on GPSIMD, rest on DVE.
ACT_DEQ = 6
GPS_DEQ = 3


def _split_multiwait(nc):
    """This container's walrus accepts only ONE sync-wait per instruction;
    Tile's tail drain (and occasionally other insts) carry several. Split
    extras into single-wait EventSemaphore insts inserted just before."""
    for fn in nc.m.functions:
        for blk in fn.blocks:
            out = []
            for ins in blk.instructions:
                si = ins.sync_info
                if si is not None and si.on_wait is not None and len(si.on_wait) > 1:
                    waits = list(si.on_wait)
                    for j, w in enumerate(waits[:-1]):
                        out.append(mybir.InstEventSemaphore(
                            name=f"{ins.name}_sw{j}", ins=[], outs=[],
                            engine=ins.engine,
                            sync_info=mybir.SyncInfo(on_wait=[w], on_update=[])))
                    si.on_wait = [waits[-1]]
                    ins.sync_info = si
                out.append(ins)
            blk.instructions = out


def _build(start_pos: int):
    """Per core: xk/xv (B,HC,L,D) f32, prefix packed caches (B,HC,S,64) u8
    and prefix scale/zero rows (B,HC,S) f32 -> ok/ov (B,HC,S+L,D) f32."""
    S = start_pos
    E = S + L
    CP = S // 128  # prefix row-chunks per head
    assert L % 128 == 0 and S % 128 == 0 and E <= MAX_SEQ

    nc = bass.Bass(trn_type="TRN2")

    ins_q, ins_p, ins_sc, ins_zp, outs = {}, {}, {}, {}, {}
    for t in ("k", "v"):
        ins_q[t] = nc.dram_tensor(f"x{t}", [B, HC, L, D], F32, kind="ExternalInput")
        if S:
            ins_p[t] = nc.dram_tensor(f"p{t}", [B, HC, S, D // 2], U8, kind="ExternalInput")
            ins_sc[t] = nc.dram_tensor(f"sc{t}", [B, HC, S], F32, kind="ExternalInput")
            ins_zp[t] = nc.dram_tensor(f"zp{t}", [B, HC, S], F32, kind="ExternalInput")
        outs[t] = nc.dram_tensor(f"o{t}", [B, HC, E, D], F32, kind="ExternalOutput")

    units = [(b, hh, t) for b in range(B) for hh in range(HC) for t in ("k", "v")]
    NU = len(units)
    PC = B * HC * CP  # hoisted scale/zero columns per tensor

    with tile.TileContext(nc) as tc:
        with tc.tile_pool(name="big", bufs=4) as big, \
             tc.tile_pool(name="deep", bufs=6) as deep, \
             tc.tile_pool(name="deep5", bufs=6) as deep5, \
             tc.tile_pool(name="small", bufs=5) as small, \
             tc.tile_pool(name="xpool", bufs=5) as xpool:

            # ---- hoisted prefix scale/zero loads + -zero*scale (keeps the
            #      per-unit DMA count at 4 so the 8 hw-DMA lanes recycle
            #      across 2 units of prefetch depth) ----
            if S:
                scp = small.tile([128, 2 * PC], F32, tag="scp", name="scp")
                zpp = small.tile([128, 2 * PC], F32, tag="zpp", name="zpp")
                for kv, t in enumerate(("k", "v")):
                    cs = slice(kv * PC, (kv + 1) * PC)
                    nc.sync.dma_start(
                        out=scp[:, cs].rearrange("p (b hh c) -> p b hh c", b=B, hh=HC),
                        in_=ins_sc[t][:, :, :].rearrange("b hh (p c) -> p b hh c", p=128))
                    nc.sync.dma_start(
                        out=zpp[:, cs].rearrange("p (b hh c) -> p b hh c", b=B, hh=HC),
                        in_=ins_zp[t][:, :, :].rearrange("b hh (p c) -> p b hh c", p=128))
                nzs = small.tile([128, 2 * PC], F32, tag="nzs", name="nzs")
                nc.vector.tensor_tensor(out=nzs[:, :], in0=zpp[:, :], in1=scp[:, :],
                                        op=Alu.mult)
                nc.vector.tensor_scalar(out=nzs[:, :], in0=nzs[:, :], scalar1=-1.0,
                                        scalar2=None, op0=Alu.mult)

            st = [None] * NU  # per-unit live tiles for the delayed dequant/store

            def emit_deq(u, engines=("act", "gps", "dve")):
                """Dequant quant-region of unit u (q -> oq)."""
                d = st[u]
                q, oq, scl, mn = d["q"], d["oq"], d["scl"], d["mn"]
                if "act" in engines:
                    for cc in range(ACT_DEQ):
                        nc.scalar.activation(out=oq[:, cc, :], in_=q[:, cc, :],
                                             func=Act.Identity,
                                             bias=mn[:, cc:cc + 1],
                                             scale=scl[:, cc:cc + 1])
                if "gps" in engines:
                    for cc in range(ACT_DEQ, ACT_DEQ + GPS_DEQ):
                        nc.gpsimd.tensor_scalar(out=oq[:, cc, :], in0=q[:, cc, :],
                                                scalar1=scl[:, cc:cc + 1],
                                                scalar2=mn[:, cc:cc + 1],
                                                op0=Alu.mult, op1=Alu.add)
                if "dve" in engines:
                    for cc in range(ACT_DEQ + GPS_DEQ, CQ):
                        nc.vector.tensor_scalar(out=oq[:, cc, :], in0=q[:, cc, :],
                                                scalar1=scl[:, cc:cc + 1],
                                                scalar2=mn[:, cc:cc + 1],
                                                op0=Alu.mult, op1=Alu.add)

            def emit_store(u):
                """DMA unit u's quant-region output out (issued 2 units later
                so the descriptor walk on the sync queue never waits)."""
                b, hh, t = units[u]
                oq_dram = outs[t][b, hh, S:E, :].rearrange("(p c) d -> p c d", p=128)
                nc.sync.dma_start(out=oq_dram, in_=st[u]["oq"][:, :, :])
                st[u] = None

            def emit_store_pre(u):
                b, hh, t = units[u]
                op_dram = outs[t][b, hh, 0:S, :].rearrange("(p c) d -> p c d", p=128)
                nc.sync.dma_start(out=op_dram, in_=st[u]["opre"][:, :, :])

            for u, (b, hh, t) in enumerate(units):
                # ---- DMA in ----
                x = xpool.tile([128, CQ, D], F32, tag="x")
                nc.sync.dma_start(
                    out=x[:, :, :],
                    in_=ins_q[t][b, hh, :, :].rearrange("(p c) d -> p c d", p=128))
                if S:
                    pkt = big.tile([128, CP, D // 2], U8, tag="pkt")
                    nc.sync.dma_start(
                        out=pkt[:, :, :],
                        in_=ins_p[t][b, hh, :, :].rearrange("(p c) d -> p c d", p=128))

                # ---- delayed output walks (data long since ready; the
                #      delay keeps the walks' data-waits satisfied so the
                #      input walks queued behind them never stall) ----
                if u > 2 and S:
                    emit_store_pre(u - 3)
                if u > 3:
                    emit_store(u - 4)

                # ---- ACT: dequant share (2 units back: q is complete, so
                #      no engine ever waits on the quant stream) ----
                if u > 1:
                    emit_deq(u - 2, engines=("act",))

                # ---- DVE: nibble unpack first (unblocks GPSIMD) ----
                if S:
                    lohi = big.tile([128, CP, D], U8, tag="lohi")
                    pk32 = pkt[:, :, :].bitcast(U32)
                    nc.vector.tensor_scalar(out=lohi[:, :, 0:D // 2].bitcast(U32),
                                            in0=pk32, scalar1=0x0F0F0F0F, scalar2=None,
                                            op0=Alu.bitwise_and)
                    nc.vector.tensor_scalar(out=lohi[:, :, D // 2:D].bitcast(U32),
                                            in0=pk32, scalar1=4, scalar2=0x0F0F0F0F,
                                            op0=Alu.logical_shift_right,
                                            op1=Alu.bitwise_and)

                # ---- DVE/GPSIMD: dequant shares (2 units back) ----
                if u > 1:
                    emit_deq(u - 2, engines=("gps",))
                    emit_deq(u - 2, engines=("dve",))

                # ---- GPSIMD: prefix dequant (u8 in, interleaved out) ----
                if S:
                    pcol = (b * HC + hh) * CP + (0 if t == "k" else PC)
                    opre = deep.tile([128, CP, D], F32, tag="opre")
                    for cc in range(CP):
                        src = lohi[:, cc, :].rearrange("p (two d) -> p two d", two=2)
                        dst = opre[:, cc, :].rearrange("p (d two) -> p two d", two=2)
                        pc = pcol + cc
                        nc.gpsimd.tensor_scalar(out=dst, in0=src,
                                                scalar1=scp[:, pc:pc + 1],
                                                scalar2=nzs[:, pc:pc + 1],
                                                op0=Alu.mult, op1=Alu.add)

                # ---- DVE: min/max + stats chain ----
                mx = small.tile([128, CQ], F32, tag="mx")
                mn = small.tile([128, CQ], F32, tag="mn")
                nc.vector.tensor_reduce(out=mx[:, :], in_=x[:, :, :], axis=AX.X, op=Alu.max)
                nc.vector.tensor_reduce(out=mn[:, :], in_=x[:, :, :], axis=AX.X, op=Alu.min)
                scl = small.tile([128, CQ], F32, tag="scl")
                nc.vector.tensor_tensor(out=scl[:, :], in0=mx[:, :], in1=mn[:, :], op=Alu.subtract)
                nc.vector.tensor_scalar(out=scl[:, :], in0=scl[:, :], scalar1=INV15,
                                        scalar2=1e-8, op0=Alu.mult, op1=Alu.max)
                rcp = small.tile([128, CQ], F32, tag="rcp")
                nc.vector.reciprocal(out=rcp[:, :], in_=scl[:, :])
                zero = small.tile([128, CQ], F32, tag="zero")
                nc.vector.tensor_scalar(out=zero[:, :], in0=mn[:, :], scalar1=-1.0,
                                        scalar2=None, op0=Alu.mult)
                nc.vector.tensor_tensor(out=zero[:, :], in0=zero[:, :], in1=rcp[:, :], op=Alu.mult)

                # ---- ACT: quantize (f32 -> u8 is an RNE round) ----
                q = deep5.tile([128, CQ, D], U8, tag="q")
                for cc in range(CQ):
                    nc.scalar.activation(out=q[:, cc, :], in_=x[:, cc, :],
                                         func=Act.Identity,
                                         bias=zero[:, cc:cc + 1],
                                         scale=rcp[:, cc:cc + 1])

                oq = deep5.tile([128, CQ, D], F32, tag="oq", name="oq")
                st[u] = {"q": q, "oq": oq, "scl": scl, "mn": mn,
                         "opre": opre if S else None}

            if S:
                emit_store_pre(NU - 3)
                emit_store_pre(NU - 2)
                emit_store_pre(NU - 1)
            emit_deq(NU - 2)
            emit_store(NU - 4)
            emit_store(NU - 3)
            emit_deq(NU - 1)
            emit_store(NU - 2)
            emit_store(NU - 1)

    _split_multiwait(nc)
    return nc


_CACHE = {}


def _get_nc(start_pos: int):
    if start_pos not in _CACHE:
        _CACHE[start_pos] = _build(start_pos)
    return _CACHE[start_pos]


def _install_ntff_hook_shim():
    """The agent image's antenv lacks axon_hooks; recreate it so
    run_bass_kernel_spmd(trace=True) can drive NTFF profiling."""
    import types
    if "antenv.axon_hooks" in sys.modules:
        return
    mod = types.ModuleType("antenv.axon_hooks")
    state = {"hook": None}
    try:
        from trn_agent_boot.trn_boot import _ntff_profile_via_ctypes
        state["hook"] = _ntff_profile_via_ctypes("/opt/axon/libaxon_pjrt.so")
    except Exception:
        pass
    mod.get_axon_ntff_profile_hook = lambda: state["hook"]
    mod.set_axon_ntff_profile_hook = lambda h: state.__setitem__("hook", h)
    sys.modules["antenv.axon_hooks"] = mod


def _kernel_np(k, v, k_cache, v_cache, k_scale, k_zero, v_scale, v_zero, start_pos):
    """Pure-numpy fallback for shapes the bass path doesn't handle."""
    def qp(x):
        mn = x.min(-1, keepdims=True)
        mx = x.max(-1, keepdims=True)
        scale = np.maximum((mx - mn) / np.float32(15.0), np.float32(1e-8))
        zero = -mn / scale
        q = np.clip(np.round(x / scale + zero), 0, 15).astype(np.uint8)
        return (q[..., 0::2] | (q[..., 1::2] << 4)), scale[..., 0], zero[..., 0]

    def dq(p, s, z):
        lo = (p & 15).astype(np.float32)
        hi = ((p >> 4) & 15).astype(np.float32)
        q = np.stack([lo, hi], -1).reshape(p.shape[:-1] + (p.shape[-1] * 2,))
        return (q - z[..., None]) * s[..., None]

    S = int(start_pos)
    E = S + k.shape[2]
    outs = []
    for x, cache, sc, zp in ((k, k_cache, k_scale, k_zero), (v, v_cache, v_scale, v_zero)):
        pp, ps, pz = qp(x)
        cache = cache.copy(); sc = sc.copy(); zp = zp.copy()
        cache[:, :, S:E] = pp
        sc[:, :, S:E] = ps
        zp[:, :, S:E] = pz
        outs.append(dq(cache[:, :, :E], sc[:, :, :E], zp[:, :, :E]))
    return tuple(outs)


def kernel(k, v, k_cache, v_cache, k_scale, k_zero, v_scale, v_zero, start_pos,
           _trace=False):
    k = np.asarray(k, np.float32)
    v = np.asarray(v, np.float32)
    k_cache = np.asarray(k_cache, np.uint8)
    v_cache = np.asarray(v_cache, np.uint8)
    k_scale = np.asarray(k_scale, np.float32)
    k_zero = np.asarray(k_zero, np.float32)
    v_scale = np.asarray(v_scale, np.float32)
    v_zero = np.asarray(v_zero, np.float32)
    S = int(start_pos)

    if (k.shape != (B, H, L, D) or S % 128 or S + L > MAX_SEQ):
        return _kernel_np(k, v, k_cache, v_cache, k_scale, k_zero, v_scale, v_zero, S)

    nc = _get_nc(S)
    E = S + L

    in_maps = []
    for m in range(N_CORES):
        hs = slice(m * HC, (m + 1) * HC)
        im = {
            "xk": np.ascontiguousarray(k[:, hs]),
            "xv": np.ascontiguousarray(v[:, hs]),
        }
        if S:
            im["pk"] = np.ascontiguousarray(k_cache[:, hs, :S, :])
            im["pv"] = np.ascontiguousarray(v_cache[:, hs, :S, :])
            im["sck"] = np.ascontiguousarray(k_scale[:, hs, :S])
            im["zpk"] = np.ascontiguousarray(k_zero[:, hs, :S])
            im["scv"] = np.ascontiguousarray(v_scale[:, hs, :S])
            im["zpv"] = np.ascontiguousarray(v_zero[:, hs, :S])
        in_maps.append(im)

    if _trace:
        _install_ntff_hook_shim()
    res = run_bass_kernel_spmd(nc, in_maps, list(range(N_CORES)), trace=_trace)

    k_dec = np.empty((B, H, E, D), np.float32)
    v_dec = np.empty((B, H, E, D), np.float32)
    for m in range(N_CORES):
        hs = slice(m * HC, (m + 1) * HC)
        k_dec[:, hs] = res.results[m]["ok"]
        v_dec[:, hs] = res.results[m]["ov"]
    if _trace:
        return (k_dec, v_dec), res
    return k_dec, v_dec


# revision 32
# speedup vs baseline: 1.0070x; 1.0070x over previous
"""CompressedKVCache kernel for Trainium2 (8 NeuronCores, head-sharded).

Per (b, h) head: quantize k/v rows to int4 (per-row min/max affine),
then return the dequantized cache prefix [0, start+L): rows [0, start)
decoded from the packed uint8 cache inputs, rows [start, start+L)
quantize->dequantized entirely on-chip.

Sharding: H=32 heads split across 8 cores (4 heads each); fully
independent per head, no cross-core communication.

Layout: row-block-per-partition ("(p c) d") so every DMA descriptor is
1-8KB contiguous (a "(c p) d" layout produces 512B descriptors and
leaves the DMA engines descriptor-rate-bound at ~47% HBM utilization).

Engine split (measured per-op costs):
  DVE   - min/max 3D reduces, stats chain, nibble unpack, 11/16 of the
          quant-region dequant chunks (tensor_scalar, ~314ns/chunk)
  ACT   - all quantize chunks (Identity, scale/bias APs, i32 out = RNE
          round for free; 387ns/chunk) + 5/16 of dequant chunks
  GPSIMD- all prefix dequant chunks (u8 in, strided interleave out,
          ~445ns/chunk). NOTE: gpsimd mis-executes i32 inputs; only u8
          inputs are used here.

Work is pipelined per tensor-unit (B*HC*2 = 16 units of 2048 rows):
unit u's dequant is emitted during unit u+1 so no engine stalls on the
ACT quant stream; prefix outputs DMA out as soon as GPSIMD finishes.
Prefix scale/zero rows are hoisted into one DMA per input tensor.
"""

import sys

sys.path.insert(0, "/opt/trn_rl_repo")

import numpy as np
from concourse import bass, mybir
from concourse import tile
from concourse.bass_utils import run_bass_kernel_spmd

F32 = mybir.dt.float32
U8 = mybir.dt.uint8
U32 = mybir.dt.uint32
I32 = mybir.dt.int32
Alu = mybir.AluOpType
Act = mybir.ActivationFunctionType
AX = mybir.AxisListType
INV15 = float(np.float32(1.0 / 15.0))

B, H, L, D = 2, 32, 2048, 128
MAX_SEQ = 8192
N_CORES = 8
HC = H // N_CORES  # heads per core
CQ = L // 128      # quant row-chunks per head (16)

# Per-unit dequant chunk split across engines (of CQ=16 chunks):
# first ACT_DEQ on ACT, next GPS_DEQ on GPSIMD, rest on DVE.
ACT_DEQ = 6
GPS_DEQ = 3


def _split_multiwait(nc):
    """This container's walrus accepts only ONE sync-wait per instruction;
    Tile's tail drain (and occasionally other insts) carry several. Split
    extras into single-wait EventSemaphore insts inserted just before."""
    for fn in nc.m.functions:
        for blk in fn.blocks:
            out = []
            for ins in blk.instructions:
                si = ins.sync_info
                if si is not None and si.on_wait is not None and len(si.on_wait) > 1:
                    waits = list(si.on_wait)
                    for j, w in enumerate(waits[:-1]):
                        out.append(mybir.InstEventSemaphore(
                            name=f"{ins.name}_sw{j}", ins=[], outs=[],
                            engine=ins.engine,
                            sync_info=mybir.SyncInfo(on_wait=[w], on_update=[])))
                    si.on_wait = [waits[-1]]
                    ins.sync_info = si
                out.append(ins)
            blk.instructions = out


def _build(start_pos: int):
    """Per core: xk/xv (B,HC,L,D) f32, prefix packed caches (B,HC,S,64) u8
    and prefix scale/zero rows (B,HC,S) f32 -> ok/ov (B,HC,S+L,D) f32."""
    S = start_pos
    E = S + L
    CP = S // 128  # prefix row-chunks per head
    assert L % 128 == 0 and S % 128 == 0 and E <= MAX_SEQ

    nc = bass.Bass(trn_type="TRN2")

    ins_q, ins_p, ins_sc, ins_zp, outs = {}, {}, {}, {}, {}
    for t in ("k", "v"):
        ins_q[t] = nc.dram_tensor(f"x{t}", [B, HC, L, D], F32, kind="ExternalInput")
        if S:
            ins_p[t] = nc.dram_tensor(f"p{t}", [B, HC, S, D // 2], U8, kind="ExternalInput")
            ins_sc[t] = nc.dram_tensor(f"sc{t}", [B, HC, S], F32, kind="ExternalInput")
            ins_zp[t] = nc.dram_tensor(f"zp{t}", [B, HC, S], F32, kind="ExternalInput")
        outs[t] = nc.dram_tensor(f"o{t}", [B, HC, E, D], F32, kind="ExternalOutput")

    units = [(b, hh, t) for b in range(B) for hh in range(HC) for t in ("k", "v")]
    NU = len(units)
    PC = B * HC * CP  # hoisted scale/zero columns per tensor

    with tile.TileContext(nc) as tc:
        with tc.tile_pool(name="big", bufs=4) as big, \
             tc.tile_pool(name="deep", bufs=6) as deep, \
             tc.tile_pool(name="deep5", bufs=6) as deep5, \
             tc.tile_pool(name="small", bufs=5) as small, \
             tc.tile_pool(name="xpool", bufs=5) as xpool:

            # ---- hoisted prefix scale/zero loads + -zero*scale (keeps the
            #      per-unit DMA count at 4 so the 8 hw-DMA lanes recycle
            #      across 2 units of prefetch depth) ----
            if S:
                scp = small.tile([128, 2 * PC], F32, tag="scp", name="scp")
                zpp = small.tile([128, 2 * PC], F32, tag="zpp", name="zpp")
                for kv, t in enumerate(("k", "v")):
                    cs = slice(kv * PC, (kv + 1) * PC)
                    nc.sync.dma_start(
                        out=scp[:, cs].rearrange("p (b hh c) -> p b hh c", b=B, hh=HC),
                        in_=ins_sc[t][:, :, :].rearrange("b hh (p c) -> p b hh c", p=128))
                    nc.sync.dma_start(
                        out=zpp[:, cs].rearrange("p (b hh c) -> p b hh c", b=B, hh=HC),
                        in_=ins_zp[t][:, :, :].rearrange("b hh (p c) -> p b hh c", p=128))
                nzs = small.tile([128, 2 * PC], F32, tag="nzs", name="nzs")
                nc.vector.tensor_tensor(out=nzs[:, :], in0=zpp[:, :], in1=scp[:, :],
                                        op=Alu.mult)
                nc.vector.tensor_scalar(out=nzs[:, :], in0=nzs[:, :], scalar1=-1.0,
                                        scalar2=None, op0=Alu.mult)

            st = [None] * NU  # per-unit live tiles for the delayed dequant/store

            def emit_deq(u, engines=("act", "gps", "dve")):
                """Dequant quant-region of unit u (q -> oq)."""
                d = st[u]
                q, oq, scl, mn = d["q"], d["oq"], d["scl"], d["mn"]
                if "act" in engines:
                    for cc in range(ACT_DEQ):
                        nc.scalar.activation(out=oq[:, cc, :], in_=q[:, cc, :],
                                             func=Act.Identity,
                                             bias=mn[:, cc:cc + 1],
                                             scale=scl[:, cc:cc + 1])
                if "gps" in engines:
                    for cc in range(ACT_DEQ, ACT_DEQ + GPS_DEQ):
                        nc.gpsimd.tensor_scalar(out=oq[:, cc, :], in0=q[:, cc, :],
                                                scalar1=scl[:, cc:cc + 1],
                                                scalar2=mn[:, cc:cc + 1],
                                                op0=Alu.mult, op1=Alu.add)
                if "dve" in engines:
                    for cc in range(ACT_DEQ + GPS_DEQ, CQ):
                        nc.vector.tensor_scalar(out=oq[:, cc, :], in0=q[:, cc, :],
                                                scalar1=scl[:, cc:cc + 1],
                                                scalar2=mn[:, cc:cc + 1],
                                                op0=Alu.mult, op1=Alu.add)

            def emit_store(u):
                """DMA unit u's quant-region output out (issued 2 units later
                so the descriptor walk on the sync queue never waits)."""
                b, hh, t = units[u]
                oq_dram = outs[t][b, hh, S:E, :].rearrange("(p c) d -> p c d", p=128)
                nc.sync.dma_start(out=oq_dram, in_=st[u]["oq"][:, :, :])
                st[u] = None

            def emit_store_pre(u):
                b, hh, t = units[u]
                op_dram = outs[t][b, hh, 0:S, :].rearrange("(p c) d -> p c d", p=128)
                nc.sync.dma_start(out=op_dram, in_=st[u]["opre"][:, :, :])

            for u, (b, hh, t) in enumerate(units):
                # ---- DMA in ----
                x = xpool.tile([128, CQ, D], F32, tag="x")
                nc.sync.dma_start(
                    out=x[:, :, :],
                    in_=ins_q[t][b, hh, :, :].rearrange("(p c) d -> p c d", p=128))
                if S:
                    pkt = big.tile([128, CP, D // 2], U8, tag="pkt")
                    nc.sync.dma_start(
                        out=pkt[:, :, :],
                        in_=ins_p[t][b, hh, :, :].rearrange("(p c) d -> p c d", p=128))

                # ---- delayed output walks (data long since ready; the
                #      delay keeps the walks' data-waits satisfied so the
                #      input walks queued behind them never stall) ----
                if u > 3 and S:
                    emit_store_pre(u - 4)
                if u > 4:
                    emit_store(u - 5)

                # ---- ACT: dequant share (2 units back: q is complete, so
                #      no engine ever waits on the quant stream) ----
                if u > 1:
                    emit_deq(u - 2, engines=("act",))

                # ---- DVE: nibble unpack first (unblocks GPSIMD) ----
                if S:
                    lohi = big.tile([128, CP, D], U8, tag="lohi")
                    pk32 = pkt[:, :, :].bitcast(U32)
                    nc.vector.tensor_scalar(out=lohi[:, :, 0:D // 2].bitcast(U32),
                                            in0=pk32, scalar1=0x0F0F0F0F, scalar2=None,
                                            op0=Alu.bitwise_and)
                    nc.vector.tensor_scalar(out=lohi[:, :, D // 2:D].bitcast(U32),
                                            in0=pk32, scalar1=4, scalar2=0x0F0F0F0F,
                                            op0=Alu.logical_shift_right,
                                            op1=Alu.bitwise_and)

                # ---- DVE/GPSIMD: dequant shares (2 units back) ----
                if u > 1:
                    emit_deq(u - 2, engines=("gps",))
                    emit_deq(u - 2, engines=("dve",))

                # ---- GPSIMD: prefix dequant (u8 in, interleaved out) ----
                if S:
                    pcol = (b * HC + hh) * CP + (0 if t == "k" else PC)
                    opre = deep.tile([128, CP, D], F32, tag="opre")
                    for cc in range(CP):
                        src = lohi[:, cc, :].rearrange("p (two d) -> p two d", two=2)
                        dst = opre[:, cc, :].rearrange("p (d two) -> p two d", two=2)
                        pc = pcol + cc
                        nc.gpsimd.tensor_scalar(out=dst, in0=src,
                                                scalar1=scp[:, pc:pc + 1],
                                                scalar2=nzs[:, pc:pc + 1],
                                                op0=Alu.mult, op1=Alu.add)

                # ---- DVE: min/max + stats chain ----
                mx = small.tile([128, CQ], F32, tag="mx")
                mn = small.tile([128, CQ], F32, tag="mn")
                nc.vector.tensor_reduce(out=mx[:, :], in_=x[:, :, :], axis=AX.X, op=Alu.max)
                nc.vector.tensor_reduce(out=mn[:, :], in_=x[:, :, :], axis=AX.X, op=Alu.min)
                scl = small.tile([128, CQ], F32, tag="scl")
                nc.vector.tensor_tensor(out=scl[:, :], in0=mx[:, :], in1=mn[:, :], op=Alu.subtract)
                nc.vector.tensor_scalar(out=scl[:, :], in0=scl[:, :], scalar1=INV15,
                                        scalar2=1e-8, op0=Alu.mult, op1=Alu.max)
                rcp = small.tile([128, CQ], F32, tag="rcp")
                nc.vector.reciprocal(out=rcp[:, :], in_=scl[:, :])
                zero = small.tile([128, CQ], F32, tag="zero")
                nc.vector.tensor_scalar(out=zero[:, :], in0=mn[:, :], scalar1=-1.0,
                                        scalar2=None, op0=Alu.mult)
                nc.vector.tensor_tensor(out=zero[:, :], in0=zero[:, :], in1=rcp[:, :], op=Alu.mult)

                # ---- ACT: quantize (f32 -> u8 is an RNE round) ----
                q = deep5.tile([128, CQ, D], U8, tag="q")
                for cc in range(CQ):
                    nc.scalar.activation(out=q[:, cc, :], in_=x[:, cc, :],
                                         func=Act.Identity,
                                         bias=zero[:, cc:cc + 1],
                                         scale=rcp[:, cc:cc + 1])

                oq = deep5.tile([128, CQ, D], F32, tag="oq", name="oq")
                st[u] = {"q": q, "oq": oq, "scl": scl, "mn": mn,
                         "opre": opre if S else None}

            if S:
                emit_store_pre(NU - 4)
                emit_store_pre(NU - 3)
                emit_store_pre(NU - 2)
                emit_store_pre(NU - 1)
            emit_deq(NU - 2)
            emit_store(NU - 5)
            emit_store(NU - 4)
            emit_store(NU - 3)
            emit_deq(NU - 1)
            emit_store(NU - 2)
            emit_store(NU - 1)

    _split_multiwait(nc)
    return nc


_CACHE = {}


def _get_nc(start_pos: int):
    if start_pos not in _CACHE:
        _CACHE[start_pos] = _build(start_pos)
    return _CACHE[start_pos]


def _install_ntff_hook_shim():
    """The agent image's antenv lacks axon_hooks; recreate it so
    run_bass_kernel_spmd(trace=True) can drive NTFF profiling."""
    import types
    if "antenv.axon_hooks" in sys.modules:
        return
    mod = types.ModuleType("antenv.axon_hooks")
    state = {"hook": None}
    try:
        from trn_agent_boot.trn_boot import _ntff_profile_via_ctypes
        state["hook"] = _ntff_profile_via_ctypes("/opt/axon/libaxon_pjrt.so")
    except Exception:
        pass
    mod.get_axon_ntff_profile_hook = lambda: state["hook"]
    mod.set_axon_ntff_profile_hook = lambda h: state.__setitem__("hook", h)
    sys.modules["antenv.axon_hooks"] = mod


def _kernel_np(k, v, k_cache, v_cache, k_scale, k_zero, v_scale, v_zero, start_pos):
    """Pure-numpy fallback for shapes the bass path doesn't handle."""
    def qp(x):
        mn = x.min(-1, keepdims=True)
        mx = x.max(-1, keepdims=True)
        scale = np.maximum((mx - mn) / np.float32(15.0), np.float32(1e-8))
        zero = -mn / scale
        q = np.clip(np.round(x / scale + zero), 0, 15).astype(np.uint8)
        return (q[..., 0::2] | (q[..., 1::2] << 4)), scale[..., 0], zero[..., 0]

    def dq(p, s, z):
        lo = (p & 15).astype(np.float32)
        hi = ((p >> 4) & 15).astype(np.float32)
        q = np.stack([lo, hi], -1).reshape(p.shape[:-1] + (p.shape[-1] * 2,))
        return (q - z[..., None]) * s[..., None]

    S = int(start_pos)
    E = S + k.shape[2]
    outs = []
    for x, cache, sc, zp in ((k, k_cache, k_scale, k_zero), (v, v_cache, v_scale, v_zero)):
        pp, ps, pz = qp(x)
        cache = cache.copy(); sc = sc.copy(); zp = zp.copy()
        cache[:, :, S:E] = pp
        sc[:, :, S:E] = ps
        zp[:, :, S:E] = pz
        outs.append(dq(cache[:, :, :E], sc[:, :, :E], zp[:, :, :E]))
    return tuple(outs)


def kernel(k, v, k_cache, v_cache, k_scale, k_zero, v_scale, v_zero, start_pos,
           _trace=False):
    k = np.asarray(k, np.float32)
    v = np.asarray(v, np.float32)
    k_cache = np.asarray(k_cache, np.uint8)
    v_cache = np.asarray(v_cache, np.uint8)
    k_scale = np.asarray(k_scale, np.float32)
    k_zero = np.asarray(k_zero, np.float32)
    v_scale = np.asarray(v_scale, np.float32)
    v_zero = np.asarray(v_zero, np.float32)
    S = int(start_pos)

    if (k.shape != (B, H, L, D) or S % 128 or S + L > MAX_SEQ):
        return _kernel_np(k, v, k_cache, v_cache, k_scale, k_zero, v_scale, v_zero, S)

    nc = _get_nc(S)
    E = S + L

    in_maps = []
    for m in range(N_CORES):
        hs = slice(m * HC, (m + 1) * HC)
        im = {
            "xk": np.ascontiguousarray(k[:, hs]),
            "xv": np.ascontiguousarray(v[:, hs]),
        }
        if S:
            im["pk"] = np.ascontiguousarray(k_cache[:, hs, :S, :])
            im["pv"] = np.ascontiguousarray(v_cache[:, hs, :S, :])
            im["sck"] = np.ascontiguousarray(k_scale[:, hs, :S])
            im["zpk"] = np.ascontiguousarray(k_zero[:, hs, :S])
            im["scv"] = np.ascontiguousarray(v_scale[:, hs, :S])
            im["zpv"] = np.ascontiguousarray(v_zero[:, hs, :S])
        in_maps.append(im)

    if _trace:
        _install_ntff_hook_shim()
    res = run_bass_kernel_spmd(nc, in_maps, list(range(N_CORES)), trace=_trace)

    k_dec = np.empty((B, H, E, D), np.float32)
    v_dec = np.empty((B, H, E, D), np.float32)
    for m in range(N_CORES):
        hs = slice(m * HC, (m + 1) * HC)
        k_dec[:, hs] = res.results[m]["ok"]
        v_dec[:, hs] = res.results[m]["ov"]
    if _trace:
        return (k_dec, v_dec), res
    return k_dec, v_dec
